# revision 21
# baseline (speedup 1.0000x reference)
"""Distributed Trainium2 kernel for the attention GEMV chain:

    score = context_vector @ query            [L]         (L=8192, Q=4096)
    attn  = softmax(score)
    s_t   = attn @ context_vector             [Q]
    out   = K_w @ concat(query, s_t)          [Q]

Sharding over 8 NeuronCores:
  - context_vector rows: 1024 per core (score GEMV + partial weighted sums)
  - K_w rows: 512 per core (each core produces its own slice of the final
    output, so no output collective is needed)
  - flash-softmax partials are accumulated per GROUP of 3 cv tiles (group
    max as exp reference keeps exp <= 1, always fp32-safe); one AllGather
    moves the 3 bf16 group rows [s_t_grp(4096) | max | expsum] per core
    (fp32 stats ride along bit-cast into the bf16 payload); the global
    normalization finishes after the gather with an alpha-weighted rank-24
    bf16 matmul that also broadcasts s_t to all 128 partitions (in PSUM).

Per-core schedule (paced by the DMA stream):
  - query arrives pre-broadcast [128, 4096] from the host shard prep and
    loads on the scalar DMA ring, in parallel with cv tile 0 on sync.
  - per 128-row cv tile: fused mult+reduce (scalar_tensor_tensor) gives the
    128 scores in one DVE pass; the tile is cast to bf16 (alternating
    DVE/ACT); per group one gpsimd partition_all_reduce gives the maxes;
    ACT computes bf16 exp weights; TensorE accumulates the exp-weighted
    rows into the group's PSUM row (partitions {0,32,64}) with cheap bf16
    matmuls; one ACT copy per group stages the row (already bf16).
  - scores stay fp32 end-to-end (softmax is argmax-dominated); the weights
    and matrices tolerate bf16 (verified ~1e-3 rel err).
  - all SBUF pools are laid out disjoint (no stack reuse), so the K_w
    stream flows right behind the cv stream with no address-reuse stalls;
    K_w streams as 8 half-tiles [128, 4096], the query-half dot products
    run before/during the collective, the s_t-half ones read the broadcast
    s_t directly from PSUM.
"""
import sys

if "/opt/trn_rl_repo" not in sys.path:
    sys.path.insert(0, "/opt/trn_rl_repo")

from contextlib import ExitStack

import numpy as np

import concourse.bass as bass
import concourse.bacc as bacc
import concourse.mybir as mybir
import concourse.tile as tile
from concourse.bass_isa import ReduceOp
from concourse.bass_utils import run_bass_kernel_spmd

N_CORES = 8
Q = 4096
L = 8192
L_SHARD = L // N_CORES          # 1024 rows of context_vector per core
R_SHARD = Q // N_CORES          # 512 rows of K_w per core
LT = L_SHARD // 128             # 8 l-tiles per core
RT = R_SHARD // 128             # 4 r-tiles per core
NB = Q // 512                   # 8 psum banks of 512 fp32
CCW = Q + 16                    # bf16 collective row: s_t_grp, m, S, pad
GROUPS = [(0, 3), (3, 6), (6, 8)]   # cv tile groups, one PSUM row each
NG = len(GROUPS)
GROWS = N_CORES * NG            # 24 gathered rows
DT = mybir.dt.float32
BF = mybir.dt.bfloat16

_NC_CACHE = {}
_DEBUG = False


def build_nc():
    nc = bacc.Bacc("TRN2", target_bir_lowering=False, debug=False,
                   num_devices=N_CORES)

    q_ext = nc.dram_tensor("query", [128, Q], DT, kind="ExternalInput")
    cv_ext = nc.dram_tensor("cv", [L_SHARD, Q], DT, kind="ExternalInput")
    kw_ext = nc.dram_tensor("kw", [R_SHARD, 2 * Q], DT, kind="ExternalInput")
    out_ext = nc.dram_tensor("out", [128, RT], DT, kind="ExternalOutput")

    cc_in = nc.dram_tensor("cc_in", [1, NG * CCW], BF)
    cc_outA = nc.dram_tensor("cc_outA", [N_CORES, 2 * CCW], BF,
                             addr_space="Shared")
    cc_outB = nc.dram_tensor("cc_outB", [N_CORES, CCW], BF,
                             addr_space="Shared")
    dbg_ext = None
    if _DEBUG:
        dbg_ext = nc.dram_tensor("dbg", [16, 16], DT, kind="ExternalOutput")

    with tile.TileContext(nc) as tc, ExitStack() as ctx:
        persist = ctx.enter_context(tc.tile_pool(name="persist", bufs=1))
        smalls = ctx.enter_context(tc.tile_pool(name="smalls", bufs=1))
        late = ctx.enter_context(tc.tile_pool(name="late", bufs=1))

        queryB = persist.tile([128, Q], DT)
        nc.scalar.dma_start(out=queryB[:, 0:Q // 2], in_=q_ext[:, 0:Q // 2])
        nc.sync.dma_start(out=queryB[:, Q // 2:Q], in_=q_ext[:, Q // 2:Q])

        scores = smalls.tile([128, LT], DT)
        dummy = smalls.tile([128, 1], DT)
        mstack = smalls.tile([128, NG], DT)     # per-group max (replicated)
        nstack = smalls.tile([128, NG], DT)     # negated maxes
        estack = smalls.tile([128, LT], BF)     # per-tile bf16 exp weights
        sgrp = smalls.tile([128, NG], DT)       # per-group expsum
        tmp_max = smalls.tile([128, LT], DT)    # per-column partition maxes
        stage = persist.tile([128, Q], BF)      # staged bf16 rows {0,32,64}
        ones_rep = smalls.tile([GROWS, 128], BF)
        nc.vector.memset(ones_rep, 1.0)

        # ---- phase 1: stream cv; per-group scores, stats, weighted row ----
        with tc.tile_pool(name="cvp", bufs=3) as cvp, \
             tc.tile_pool(name="cvb", bufs=3) as cvb, \
             tc.tile_pool(name="ps1", bufs=1, space="PSUM") as ps1:
            psum_st = ps1.tile([128, Q], DT)
            for g, (g0, g1) in enumerate(GROUPS):
                r = 32 * g
                cvb_tiles = {}
                for t in range(g0, g1):
                    cv_t = cvp.tile([128, Q], DT)
                    nc.sync.dma_start(out=cv_t,
                                      in_=cv_ext[t * 128:(t + 1) * 128, :])
                    nc.vector.scalar_tensor_tensor(
                        out=dummy.broadcast_to([128, Q]),
                        in0=cv_t, scalar=1.0, in1=queryB,
                        op0=mybir.AluOpType.mult, op1=mybir.AluOpType.mult,
                        accum_out=scores[:, t:t + 1],
                    )
                    cvb_t = cvb.tile([128, Q], BF)
                    if t % 2 == 0:
                        nc.vector.tensor_copy(cvb_t, cv_t)
                    else:
                        nc.scalar.copy(cvb_t, cv_t)
                    cvb_tiles[t] = cvb_t
                # group stats: cross-partition max per column, group max,
                # negate, then per-tile exp weights
                nc.gpsimd.partition_all_reduce(
                    tmp_max[:, g0:g1], scores[:, g0:g1], 128, ReduceOp.max)
                nc.vector.tensor_reduce(
                    out=mstack[:, g:g + 1], in_=tmp_max[:, g0:g1],
                    axis=mybir.AxisListType.X, op=mybir.AluOpType.max)
                nc.vector.tensor_scalar_mul(
                    nstack[:, g:g + 1], mstack[:, g:g + 1], -1.0)
                for t in range(g0, g1):
                    nc.scalar.activation(
                        out=estack[:, t:t + 1], in_=scores[:, t:t + 1],
                        func=mybir.ActivationFunctionType.Exp,
                        bias=nstack[:, g:g + 1], scale=1.0)
                    for n in range(NB):
                        sl = slice(n * 512, (n + 1) * 512)
                        nc.tensor.matmul(
                            psum_st[r:r + 1, sl],
                            lhsT=estack[:, t:t + 1],
                            rhs=cvb_tiles[t][:, sl],
                            start=(t == g0), stop=(t == g1 - 1),
                            skip_group_check=True,
                        )
                # group expsum; stage the row (bf16) and ship everything
                se = smalls.tile([128, 1], DT)
                nc.vector.tensor_reduce(
                    out=se, in_=estack[:, g0:g1],
                    axis=mybir.AxisListType.X, op=mybir.AluOpType.add)
                nc.gpsimd.partition_all_reduce(
                    sgrp[:, g:g + 1], se, 128, ReduceOp.add)
                nc.scalar.copy(stage[r:r + 1, :], psum_st[r:r + 1, :])
                row_out = bass.AP(tensor=cc_in.ap().tensor, offset=g * CCW,
                                  ap=[[0, 1], [1, Q]])
                nc.scalar.dma_start(out=row_out, in_=stage[r:r + 1, :])
                nc.scalar.dma_start(
                    out=cc_in[0:1, g * CCW + Q:g * CCW + Q + 2],
                    in_=mstack[0:1, g:g + 1].bitcast(BF))
                nc.scalar.dma_start(
                    out=cc_in[0:1, g * CCW + Q + 2:g * CCW + Q + 4],
                    in_=sgrp[0:1, g:g + 1].bitcast(BF))
                if g == 1:
                    # early AllGather of groups 0+1 while group 2 computes
                    nc.gpsimd.collective_compute(
                        "AllGather",
                        mybir.AluOpType.bypass,
                        replica_groups=[list(range(N_CORES))],
                        ins=[cc_in[0:1, 0:2 * CCW].opt()],
                        outs=[cc_outA.ap().opt()],
                    )

        # ---- phase 2: K_w streams; query-half dots overlap everything ----
        accq = smalls.tile([128, RT], DT)
        accs = smalls.tile([128, RT], DT)
        acc = smalls.tile([128, RT], DT)
        kwqp = ctx.enter_context(tc.tile_pool(name="kwqp", bufs=2))
        kwsp = ctx.enter_context(tc.tile_pool(name="kwsp", bufs=2))

        def kw_pair(pool, p, col0):
            pair = pool.tile([128, 2, Q], DT)
            src = bass.AP(tensor=kw_ext.ap().tensor,
                          offset=p * 256 * 2 * Q + col0,
                          ap=[[2 * Q, 128], [128 * 2 * Q, 2], [1, Q]])
            nc.sync.dma_start(out=pair, in_=src)
            return pair

        kwq_tiles = []
        for p in range(RT // 2):
            pair = kw_pair(kwqp, p, 0)
            for h in range(2):
                j = 2 * p + h
                nc.vector.scalar_tensor_tensor(
                    out=dummy.broadcast_to([128, Q]),
                    in0=pair[:, h, :], scalar=1.0, in1=queryB,
                    op0=mybir.AluOpType.mult, op1=mybir.AluOpType.mult,
                    accum_out=accq[:, j:j + 1],
                )
        kws_tiles = []
        for p in range(RT // 2):
            pair = kw_pair(kwsp, p, Q)
            kws_tiles.extend([pair[:, 0, :], pair[:, 1, :]])

        # ---- phase 3: AllGather of group 2 (groups 0+1 gathered early) ----
        nc.gpsimd.collective_compute(
            "AllGather",
            mybir.AluOpType.bypass,
            replica_groups=[list(range(N_CORES))],
            ins=[cc_in[0:1, 2 * CCW:3 * CCW].opt()],
            outs=[cc_outB.ap().opt()],
        )
        gathered = late.tile([GROWS, CCW], BF)
        ginA = bass.AP(tensor=cc_outA.ap().tensor, offset=0,
                       ap=[[CCW, 16], [1, CCW]])
        nc.scalar.dma_start(out=gathered[0:16, :], in_=ginA)
        ginB = bass.AP(tensor=cc_outB.ap().tensor, offset=0,
                       ap=[[CCW, 8], [1, CCW]])
        nc.scalar.dma_start(out=gathered[16:24, :], in_=ginB)

        # ---- phase 4: global softmax combine, s_t broadcast into PSUM ----
        mg = gathered[:, Q:Q + 2].bitcast(DT)
        sg = gathered[:, Q + 2:Q + 4].bitcast(DT)
        mmax = smalls.tile([GROWS, 1], DT)
        nc.gpsimd.partition_all_reduce(mmax, mg, GROWS, ReduceOp.max)
        negM = smalls.tile([GROWS, 1], DT)
        nc.vector.tensor_scalar_mul(negM, mmax, -1.0)
        expm = smalls.tile([GROWS, 1], DT)
        nc.scalar.activation(out=expm, in_=mg,
                             func=mybir.ActivationFunctionType.Exp,
                             bias=negM, scale=1.0)
        w = smalls.tile([GROWS, 1], DT)
        nc.vector.tensor_mul(w, expm, sg)
        wsum = smalls.tile([GROWS, 1], DT)
        nc.gpsimd.partition_all_reduce(wsum, w, GROWS, ReduceOp.add)
        rS = smalls.tile([GROWS, 1], DT)
        nc.vector.reciprocal(rS, wsum)
        alpha = smalls.tile([GROWS, 1], DT)
        nc.vector.tensor_mul(alpha, expm, rS)
        alpha_rep = smalls.tile([GROWS, 128], BF)
        nc.vector.tensor_scalar_mul(alpha_rep, ones_rep, alpha)

        with tc.tile_pool(name="ps2", bufs=1, space="PSUM") as ps2:
            psum_stB = ps2.tile([128, Q], DT)
            for n in range(NB):
                sl = slice(n * 512, (n + 1) * 512)
                nc.tensor.matmul(
                    psum_stB[:, sl],
                    lhsT=alpha_rep,
                    rhs=gathered[0:GROWS, sl],
                    start=True, stop=True,
                )

            # ---- phase 5: K_w s_t-half dots against PSUM-resident s_t ----
            for j in range(RT):
                nc.vector.scalar_tensor_tensor(
                    out=dummy.broadcast_to([128, Q]),
                    in0=kws_tiles[j], scalar=1.0, in1=psum_stB,
                    op0=mybir.AluOpType.mult, op1=mybir.AluOpType.mult,
                    accum_out=accs[:, j:j + 1],
                )

        nc.vector.tensor_add(acc, accq, accs)
        nc.sync.dma_start(out=out_ext.ap(), in_=acc)

        if _DEBUG:
            nc.sync.dma_start(out=dbg_ext[0:1, 0:NG], in_=mstack[0:1, 0:NG])
            nc.sync.dma_start(out=dbg_ext[1:2, 0:NG], in_=sgrp[0:1, 0:NG])
            nc.sync.dma_start(out=dbg_ext[2:3, 0:1], in_=mg[0:1, 0:1])
            nc.sync.dma_start(out=dbg_ext[3:4, 0:8], in_=scores[0:1, 0:LT])
            nc.sync.dma_start(out=dbg_ext[4:5, 0:1], in_=alpha[0:1, 0:1])
            nc.sync.dma_start(out=dbg_ext[5:6, 0:1], in_=wsum[0:1, 0:1])
            nc.sync.dma_start(out=dbg_ext[6:7, 0:4], in_=accq[0:1, 0:4])
            nc.sync.dma_start(out=dbg_ext[7:8, 0:4], in_=accs[0:1, 0:4])

    nc.compile()
    return nc


def get_nc():
    if "nc" not in _NC_CACHE:
        _NC_CACHE["nc"] = build_nc()
    return _NC_CACHE["nc"]


def _shard_inputs(query, context_vector, K_w):
    qb = np.ascontiguousarray(
        np.broadcast_to(np.asarray(query, dtype=np.float32).reshape(1, Q),
                        (128, Q)))
    in_maps = []
    for c in range(N_CORES):
        in_maps.append({
            "query": qb,
            "cv": np.ascontiguousarray(
                context_vector[c * L_SHARD:(c + 1) * L_SHARD], dtype=np.float32),
            "kw": np.ascontiguousarray(
                K_w[c * R_SHARD:(c + 1) * R_SHARD], dtype=np.float32),
        })
    return in_maps


def kernel(query, context_vector, K_w, _trace=False, _trace_kwargs=None):
    nc = get_nc()
    in_maps = _shard_inputs(query, context_vector, K_w)
    res = run_bass_kernel_spmd(nc, in_maps, core_ids=list(range(N_CORES)),
                               trace=_trace, **(_trace_kwargs or {}))
    out = np.concatenate(
        [np.asarray(res.results[c]["out"]).T.reshape(-1) for c in range(N_CORES)]
    ).astype(np.float32)
    if _trace:
        kernel.last_results = res
    return out


# revision 22
# speedup vs baseline: 1.0598x; 1.0598x over previous
"""Distributed Trainium2 kernel for the attention GEMV chain:

    score = context_vector @ query            [L]         (L=8192, Q=4096)
    attn  = softmax(score)
    s_t   = attn @ context_vector             [Q]
    out   = K_w @ concat(query, s_t)          [Q]

Sharding over 8 NeuronCores:
  - context_vector rows: 1024 per core (score GEMV + partial weighted sums)
  - K_w rows: 512 per core (each core produces its own slice of the final
    output, so no output collective is needed)
  - flash-softmax partials are accumulated per GROUP of 3 cv tiles (group
    max as exp reference keeps exp <= 1, always fp32-safe); one AllGather
    moves the 3 bf16 group rows [s_t_grp(4096) | max | expsum] per core
    (fp32 stats ride along bit-cast into the bf16 payload); the global
    normalization finishes after the gather with an alpha-weighted rank-24
    bf16 matmul that also broadcasts s_t to all 128 partitions (in PSUM).

Per-core schedule (paced by the DMA stream):
  - query arrives pre-broadcast [128, 4096] from the host shard prep and
    loads on the scalar DMA ring, in parallel with cv tile 0 on sync.
  - per 128-row cv tile: fused mult+reduce (scalar_tensor_tensor) gives the
    128 scores in one DVE pass; the tile is cast to bf16 (alternating
    DVE/ACT); per group one gpsimd partition_all_reduce gives the maxes;
    ACT computes bf16 exp weights; TensorE accumulates the exp-weighted
    rows into the group's PSUM row (partitions {0,32,64}) with cheap bf16
    matmuls; one ACT copy per group stages the row (already bf16).
  - scores stay fp32 end-to-end (softmax is argmax-dominated); the weights
    and matrices tolerate bf16 (verified ~1e-3 rel err).
  - all SBUF pools are laid out disjoint (no stack reuse), so the K_w
    stream flows right behind the cv stream with no address-reuse stalls;
    K_w streams as 8 half-tiles [128, 4096], the query-half dot products
    run before/during the collective, the s_t-half ones read the broadcast
    s_t directly from PSUM.
"""
import sys

if "/opt/trn_rl_repo" not in sys.path:
    sys.path.insert(0, "/opt/trn_rl_repo")

from contextlib import ExitStack

import numpy as np

import concourse.bass as bass
import concourse.bacc as bacc
import concourse.mybir as mybir
import concourse.tile as tile
from concourse.bass_isa import ReduceOp
from concourse.bass_utils import run_bass_kernel_spmd

N_CORES = 8
Q = 4096
L = 8192
L_SHARD = L // N_CORES          # 1024 rows of context_vector per core
R_SHARD = Q // N_CORES          # 512 rows of K_w per core
LT = L_SHARD // 128             # 8 l-tiles per core
RT = R_SHARD // 128             # 4 r-tiles per core
NB = Q // 512                   # 8 psum banks of 512 fp32
CCW = Q + 16                    # bf16 collective row: s_t_grp, m, S, pad
GROUPS = [(0, 3), (3, 6), (6, 8)]   # cv tile groups, one PSUM row each
NG = len(GROUPS)
GROWS = N_CORES * NG            # 24 gathered rows
DT = mybir.dt.float32
BF = mybir.dt.bfloat16

_NC_CACHE = {}
_DEBUG = False


def build_nc():
    nc = bacc.Bacc("TRN2", target_bir_lowering=False, debug=False,
                   num_devices=N_CORES)

    q_ext = nc.dram_tensor("query", [128, Q], DT, kind="ExternalInput")
    cv_ext = nc.dram_tensor("cv", [L_SHARD, Q], DT, kind="ExternalInput")
    kw_ext = nc.dram_tensor("kw", [R_SHARD, 2 * Q], DT, kind="ExternalInput")
    out_ext = nc.dram_tensor("out", [128, RT], DT, kind="ExternalOutput")

    cc_in = nc.dram_tensor("cc_in", [1, NG * CCW], BF)
    cc_outA = nc.dram_tensor("cc_outA", [N_CORES, 2 * CCW], BF,
                             addr_space="Shared")
    cc_outB = nc.dram_tensor("cc_outB", [N_CORES, CCW], BF,
                             addr_space="Shared")
    dbg_ext = None
    if _DEBUG:
        dbg_ext = nc.dram_tensor("dbg", [16, 16], DT, kind="ExternalOutput")

    with tile.TileContext(nc) as tc, ExitStack() as ctx:
        persist = ctx.enter_context(tc.tile_pool(name="persist", bufs=1))
        smalls = ctx.enter_context(tc.tile_pool(name="smalls", bufs=1))
        late = ctx.enter_context(tc.tile_pool(name="late", bufs=1))

        queryB = persist.tile([128, Q], DT)
        nc.scalar.dma_start(out=queryB[:, 0:Q // 2], in_=q_ext[:, 0:Q // 2])
        nc.sync.dma_start(out=queryB[:, Q // 2:Q], in_=q_ext[:, Q // 2:Q])

        scores = smalls.tile([128, LT], DT)
        dummy = smalls.tile([128, 1], DT)
        mstack = smalls.tile([128, NG], DT)     # per-group max (replicated)
        nstack = smalls.tile([128, NG], DT)     # negated maxes
        estack = smalls.tile([128, LT], BF)     # per-tile bf16 exp weights
        sgrp = smalls.tile([128, NG], DT)       # per-group expsum
        tmp_max = smalls.tile([128, LT], DT)    # per-column partition maxes
        stage = persist.tile([128, Q], BF)      # staged bf16 rows {0,32,64}
        ones_rep = smalls.tile([GROWS, 128], BF)
        nc.vector.memset(ones_rep, 1.0)

        # ---- phase 1: stream cv; per-group scores, stats, weighted row ----
        with tc.tile_pool(name="cvp", bufs=3) as cvp, \
             tc.tile_pool(name="cvb", bufs=3) as cvb, \
             tc.tile_pool(name="ps1", bufs=1, space="PSUM") as ps1:
            psum_st = ps1.tile([128, Q], DT)
            for g, (g0, g1) in enumerate(GROUPS):
                r = 32 * g
                cvb_tiles = {}
                for t in range(g0, g1):
                    cv_t = cvp.tile([128, Q], DT)
                    nc.sync.dma_start(out=cv_t,
                                      in_=cv_ext[t * 128:(t + 1) * 128, :])
                    nc.vector.scalar_tensor_tensor(
                        out=dummy.broadcast_to([128, Q]),
                        in0=cv_t, scalar=1.0, in1=queryB,
                        op0=mybir.AluOpType.mult, op1=mybir.AluOpType.mult,
                        accum_out=scores[:, t:t + 1],
                    )
                    cvb_t = cvb.tile([128, Q], BF)
                    nc.scalar.copy(cvb_t, cv_t)
                    cvb_tiles[t] = cvb_t
                # group stats: cross-partition max per column, group max,
                # negate, then per-tile exp weights
                nc.gpsimd.partition_all_reduce(
                    tmp_max[:, g0:g1], scores[:, g0:g1], 128, ReduceOp.max)
                nc.vector.tensor_reduce(
                    out=mstack[:, g:g + 1], in_=tmp_max[:, g0:g1],
                    axis=mybir.AxisListType.X, op=mybir.AluOpType.max)
                nc.vector.tensor_scalar_mul(
                    nstack[:, g:g + 1], mstack[:, g:g + 1], -1.0)
                for t in range(g0, g1):
                    nc.scalar.activation(
                        out=estack[:, t:t + 1], in_=scores[:, t:t + 1],
                        func=mybir.ActivationFunctionType.Exp,
                        bias=nstack[:, g:g + 1], scale=1.0)
                    for n in range(NB):
                        sl = slice(n * 512, (n + 1) * 512)
                        nc.tensor.matmul(
                            psum_st[r:r + 1, sl],
                            lhsT=estack[:, t:t + 1],
                            rhs=cvb_tiles[t][:, sl],
                            start=(t == g0), stop=(t == g1 - 1),
                            skip_group_check=True,
                        )
                # group expsum; stage the row (bf16) and ship everything
                se = smalls.tile([128, 1], DT)
                nc.vector.tensor_reduce(
                    out=se, in_=estack[:, g0:g1],
                    axis=mybir.AxisListType.X, op=mybir.AluOpType.add)
                nc.gpsimd.partition_all_reduce(
                    sgrp[:, g:g + 1], se, 128, ReduceOp.add)
                nc.scalar.copy(stage[r:r + 1, :], psum_st[r:r + 1, :])
                row_out = bass.AP(tensor=cc_in.ap().tensor, offset=g * CCW,
                                  ap=[[0, 1], [1, Q]])
                nc.scalar.dma_start(out=row_out, in_=stage[r:r + 1, :])
                nc.scalar.dma_start(
                    out=cc_in[0:1, g * CCW + Q:g * CCW + Q + 2],
                    in_=mstack[0:1, g:g + 1].bitcast(BF))
                nc.scalar.dma_start(
                    out=cc_in[0:1, g * CCW + Q + 2:g * CCW + Q + 4],
                    in_=sgrp[0:1, g:g + 1].bitcast(BF))

        # ---- phase 2: K_w streams; query-half dots overlap everything ----
        accq = smalls.tile([128, RT], DT)
        accs = smalls.tile([128, RT], DT)
        acc = smalls.tile([128, RT], DT)
        kwqp = ctx.enter_context(tc.tile_pool(name="kwqp", bufs=2))
        kwsp = ctx.enter_context(tc.tile_pool(name="kwsp", bufs=2))

        def kw_pair(pool, p, col0):
            pair = pool.tile([128, 2, Q], DT)
            src = bass.AP(tensor=kw_ext.ap().tensor,
                          offset=p * 256 * 2 * Q + col0,
                          ap=[[2 * Q, 128], [128 * 2 * Q, 2], [1, Q]])
            nc.sync.dma_start(out=pair, in_=src)
            return pair

        kwq_tiles = []
        for p in range(RT // 2):
            pair = kw_pair(kwqp, p, 0)
            for h in range(2):
                j = 2 * p + h
                nc.vector.scalar_tensor_tensor(
                    out=dummy.broadcast_to([128, Q]),
                    in0=pair[:, h, :], scalar=1.0, in1=queryB,
                    op0=mybir.AluOpType.mult, op1=mybir.AluOpType.mult,
                    accum_out=accq[:, j:j + 1],
                )
        kws_tiles = []
        for p in range(RT // 2):
            pair = kw_pair(kwsp, p, Q)
            kws_tiles.extend([pair[:, 0, :], pair[:, 1, :]])

        # ---- phase 3: split AllGather (ncfw runs them back to back) ----
        nc.gpsimd.collective_compute(
            "AllGather",
            mybir.AluOpType.bypass,
            replica_groups=[list(range(N_CORES))],
            ins=[cc_in[0:1, 0:2 * CCW].opt()],
            outs=[cc_outA.ap().opt()],
        )
        nc.gpsimd.collective_compute(
            "AllGather",
            mybir.AluOpType.bypass,
            replica_groups=[list(range(N_CORES))],
            ins=[cc_in[0:1, 2 * CCW:3 * CCW].opt()],
            outs=[cc_outB.ap().opt()],
        )
        gathered = late.tile([GROWS, CCW], BF)
        ginA = bass.AP(tensor=cc_outA.ap().tensor, offset=0,
                       ap=[[CCW, 16], [1, CCW]])
        nc.scalar.dma_start(out=gathered[0:16, :], in_=ginA)
        ginB = bass.AP(tensor=cc_outB.ap().tensor, offset=0,
                       ap=[[CCW, 8], [1, CCW]])
        nc.scalar.dma_start(out=gathered[16:24, :], in_=ginB)

        # ---- phase 4: global softmax combine, s_t broadcast into PSUM ----
        mg = gathered[:, Q:Q + 2].bitcast(DT)
        sg = gathered[:, Q + 2:Q + 4].bitcast(DT)
        mmax = smalls.tile([GROWS, 1], DT)
        nc.gpsimd.partition_all_reduce(mmax, mg, GROWS, ReduceOp.max)
        negM = smalls.tile([GROWS, 1], DT)
        nc.vector.tensor_scalar_mul(negM, mmax, -1.0)
        expm = smalls.tile([GROWS, 1], DT)
        nc.scalar.activation(out=expm, in_=mg,
                             func=mybir.ActivationFunctionType.Exp,
                             bias=negM, scale=1.0)
        w = smalls.tile([GROWS, 1], DT)
        nc.vector.tensor_mul(w, expm, sg)
        wsum = smalls.tile([GROWS, 1], DT)
        nc.gpsimd.partition_all_reduce(wsum, w, GROWS, ReduceOp.add)
        rS = smalls.tile([GROWS, 1], DT)
        nc.vector.reciprocal(rS, wsum)
        alpha = smalls.tile([GROWS, 1], DT)
        nc.vector.tensor_mul(alpha, expm, rS)
        alpha_rep = smalls.tile([GROWS, 128], BF)
        nc.vector.tensor_scalar_mul(alpha_rep, ones_rep, alpha)

        with tc.tile_pool(name="ps2", bufs=1, space="PSUM") as ps2:
            psum_stB = ps2.tile([128, Q], DT)
            for n in range(NB):
                sl = slice(n * 512, (n + 1) * 512)
                nc.tensor.matmul(
                    psum_stB[:, sl],
                    lhsT=alpha_rep,
                    rhs=gathered[0:GROWS, sl],
                    start=True, stop=True,
                )

            # ---- phase 5: K_w s_t-half dots against PSUM-resident s_t ----
            for j in range(RT):
                nc.vector.scalar_tensor_tensor(
                    out=dummy.broadcast_to([128, Q]),
                    in0=kws_tiles[j], scalar=1.0, in1=psum_stB,
                    op0=mybir.AluOpType.mult, op1=mybir.AluOpType.mult,
                    accum_out=accs[:, j:j + 1],
                )

        nc.vector.tensor_add(acc, accq, accs)
        nc.sync.dma_start(out=out_ext.ap(), in_=acc)

        if _DEBUG:
            nc.sync.dma_start(out=dbg_ext[0:1, 0:NG], in_=mstack[0:1, 0:NG])
            nc.sync.dma_start(out=dbg_ext[1:2, 0:NG], in_=sgrp[0:1, 0:NG])
            nc.sync.dma_start(out=dbg_ext[2:3, 0:1], in_=mg[0:1, 0:1])
            nc.sync.dma_start(out=dbg_ext[3:4, 0:8], in_=scores[0:1, 0:LT])
            nc.sync.dma_start(out=dbg_ext[4:5, 0:1], in_=alpha[0:1, 0:1])
            nc.sync.dma_start(out=dbg_ext[5:6, 0:1], in_=wsum[0:1, 0:1])
            nc.sync.dma_start(out=dbg_ext[6:7, 0:4], in_=accq[0:1, 0:4])
            nc.sync.dma_start(out=dbg_ext[7:8, 0:4], in_=accs[0:1, 0:4])

    nc.compile()
    return nc


def get_nc():
    if "nc" not in _NC_CACHE:
        _NC_CACHE["nc"] = build_nc()
    return _NC_CACHE["nc"]


def _shard_inputs(query, context_vector, K_w):
    qb = np.ascontiguousarray(
        np.broadcast_to(np.asarray(query, dtype=np.float32).reshape(1, Q),
                        (128, Q)))
    in_maps = []
    for c in range(N_CORES):
        in_maps.append({
            "query": qb,
            "cv": np.ascontiguousarray(
                context_vector[c * L_SHARD:(c + 1) * L_SHARD], dtype=np.float32),
            "kw": np.ascontiguousarray(
                K_w[c * R_SHARD:(c + 1) * R_SHARD], dtype=np.float32),
        })
    return in_maps


def kernel(query, context_vector, K_w, _trace=False, _trace_kwargs=None):
    nc = get_nc()
    in_maps = _shard_inputs(query, context_vector, K_w)
    res = run_bass_kernel_spmd(nc, in_maps, core_ids=list(range(N_CORES)),
                               trace=_trace, **(_trace_kwargs or {}))
    out = np.concatenate(
        [np.asarray(res.results[c]["out"]).T.reshape(-1) for c in range(N_CORES)]
    ).astype(np.float32)
    if _trace:
        kernel.last_results = res
    return out


# revision 23
# speedup vs baseline: 1.1046x; 1.0423x over previous
"""Distributed Trainium2 kernel for the attention GEMV chain:

    score = context_vector @ query            [L]         (L=8192, Q=4096)
    attn  = softmax(score)
    s_t   = attn @ context_vector             [Q]
    out   = K_w @ concat(query, s_t)          [Q]

Sharding over 8 NeuronCores:
  - context_vector rows: 1024 per core (score GEMV + partial weighted sums)
  - K_w rows: 512 per core (each core produces its own slice of the final
    output, so no output collective is needed)
  - flash-softmax partials are accumulated per GROUP of 3 cv tiles (group
    max as exp reference keeps exp <= 1, always fp32-safe); one AllGather
    moves the 3 bf16 group rows [s_t_grp(4096) | max | expsum] per core
    (fp32 stats ride along bit-cast into the bf16 payload); the global
    normalization finishes after the gather with an alpha-weighted rank-24
    bf16 matmul that also broadcasts s_t to all 128 partitions (in PSUM).

Per-core schedule (paced by the DMA stream):
  - query arrives pre-broadcast [128, 4096] from the host shard prep and
    loads on the scalar DMA ring, in parallel with cv tile 0 on sync.
  - per 128-row cv tile: fused mult+reduce (scalar_tensor_tensor) gives the
    128 scores in one DVE pass; the tile is cast to bf16 (alternating
    DVE/ACT); per group one gpsimd partition_all_reduce gives the maxes;
    ACT computes bf16 exp weights; TensorE accumulates the exp-weighted
    rows into the group's PSUM row (partitions {0,32,64}) with cheap bf16
    matmuls; one ACT copy per group stages the row (already bf16).
  - scores stay fp32 end-to-end (softmax is argmax-dominated); the weights
    and matrices tolerate bf16 (verified ~1e-3 rel err).
  - all SBUF pools are laid out disjoint (no stack reuse), so the K_w
    stream flows right behind the cv stream with no address-reuse stalls;
    K_w streams as 8 half-tiles [128, 4096], the query-half dot products
    run before/during the collective, the s_t-half ones read the broadcast
    s_t directly from PSUM.
"""
import sys

if "/opt/trn_rl_repo" not in sys.path:
    sys.path.insert(0, "/opt/trn_rl_repo")

from contextlib import ExitStack

import numpy as np

import concourse.bass as bass
import concourse.bacc as bacc
import concourse.mybir as mybir
import concourse.tile as tile
from concourse.bass_isa import ReduceOp
from concourse.bass_utils import run_bass_kernel_spmd

N_CORES = 8
Q = 4096
L = 8192
L_SHARD = L // N_CORES          # 1024 rows of context_vector per core
R_SHARD = Q // N_CORES          # 512 rows of K_w per core
LT = L_SHARD // 128             # 8 l-tiles per core
RT = R_SHARD // 128             # 4 r-tiles per core
NB = Q // 512                   # 8 psum banks of 512 fp32
CCW = Q + 16                    # bf16 collective row: s_t_grp, m, S, pad
GROUPS = [(0, 3), (3, 7), (7, 8)]   # cv tile groups, one PSUM row each
NG = len(GROUPS)
GROWS = N_CORES * NG            # 24 gathered rows
DT = mybir.dt.float32
BF = mybir.dt.bfloat16

_NC_CACHE = {}
_DEBUG = False


def build_nc():
    nc = bacc.Bacc("TRN2", target_bir_lowering=False, debug=False,
                   num_devices=N_CORES)

    q_ext = nc.dram_tensor("query", [128, Q], DT, kind="ExternalInput")
    cv_ext = nc.dram_tensor("cv", [L_SHARD, Q], DT, kind="ExternalInput")
    kw_ext = nc.dram_tensor("kw", [R_SHARD, 2 * Q], DT, kind="ExternalInput")
    out_ext = nc.dram_tensor("out", [128, RT], DT, kind="ExternalOutput")

    cc_in = nc.dram_tensor("cc_in", [1, NG * CCW], BF)
    cc_outA = nc.dram_tensor("cc_outA", [N_CORES, NG * CCW], BF,
                             addr_space="Shared")
    cc_outB = nc.dram_tensor("cc_outB", [N_CORES, CCW], BF,
                             addr_space="Shared")
    dbg_ext = None
    if _DEBUG:
        dbg_ext = nc.dram_tensor("dbg", [16, 16], DT, kind="ExternalOutput")

    with tile.TileContext(nc) as tc, ExitStack() as ctx:
        persist = ctx.enter_context(tc.tile_pool(name="persist", bufs=1))
        smalls = ctx.enter_context(tc.tile_pool(name="smalls", bufs=1))
        late = ctx.enter_context(tc.tile_pool(name="late", bufs=1))

        queryB = persist.tile([128, Q], DT)
        nc.scalar.dma_start(out=queryB[:, 0:Q // 2], in_=q_ext[:, 0:Q // 2])
        nc.sync.dma_start(out=queryB[:, Q // 2:Q], in_=q_ext[:, Q // 2:Q])

        scores = smalls.tile([128, LT], DT)
        dummy = smalls.tile([128, 1], DT)
        mstack = smalls.tile([128, NG], DT)     # per-group max (replicated)
        nstack = smalls.tile([128, NG], DT)     # negated maxes
        estack = smalls.tile([128, LT], BF)     # per-tile bf16 exp weights
        sgrp = smalls.tile([128, NG], DT)       # per-group expsum
        tmp_max = smalls.tile([128, LT], DT)    # per-column partition maxes
        stage = persist.tile([128, Q], BF)      # staged bf16 rows {0,32,64}
        ones_rep = smalls.tile([GROWS, 128], BF)
        nc.vector.memset(ones_rep, 1.0)

        # ---- phase 1: stream cv; per-group scores, stats, weighted row ----
        with tc.tile_pool(name="cvp", bufs=3) as cvp, \
             tc.tile_pool(name="cvb", bufs=5) as cvb, \
             tc.tile_pool(name="ps1", bufs=1, space="PSUM") as ps1:
            psum_st = ps1.tile([128, Q], DT)
            for g, (g0, g1) in enumerate(GROUPS):
                r = 32 * g
                cvb_tiles = {}
                for t in range(g0, g1):
                    cv_t = cvp.tile([128, Q], DT)
                    nc.sync.dma_start(out=cv_t,
                                      in_=cv_ext[t * 128:(t + 1) * 128, :])
                    nc.vector.scalar_tensor_tensor(
                        out=dummy.broadcast_to([128, Q]),
                        in0=cv_t, scalar=1.0, in1=queryB,
                        op0=mybir.AluOpType.mult, op1=mybir.AluOpType.mult,
                        accum_out=scores[:, t:t + 1],
                    )
                    cvb_t = cvb.tile([128, Q], BF)
                    if t % 2 == 0:
                        nc.vector.tensor_copy(cvb_t, cv_t)
                    else:
                        nc.scalar.copy(cvb_t, cv_t)
                    cvb_tiles[t] = cvb_t
                # group stats: cross-partition max per column, group max,
                # negate, then per-tile exp weights
                nc.gpsimd.partition_all_reduce(
                    tmp_max[:, g0:g1], scores[:, g0:g1], 128, ReduceOp.max)
                nc.vector.tensor_reduce(
                    out=mstack[:, g:g + 1], in_=tmp_max[:, g0:g1],
                    axis=mybir.AxisListType.X, op=mybir.AluOpType.max)
                nc.vector.tensor_scalar_mul(
                    nstack[:, g:g + 1], mstack[:, g:g + 1], -1.0)
                for t in range(g0, g1):
                    nc.scalar.activation(
                        out=estack[:, t:t + 1], in_=scores[:, t:t + 1],
                        func=mybir.ActivationFunctionType.Exp,
                        bias=nstack[:, g:g + 1], scale=1.0)
                    for n in range(NB):
                        sl = slice(n * 512, (n + 1) * 512)
                        nc.tensor.matmul(
                            psum_st[r:r + 1, sl],
                            lhsT=estack[:, t:t + 1],
                            rhs=cvb_tiles[t][:, sl],
                            start=(t == g0), stop=(t == g1 - 1),
                            skip_group_check=True,
                        )
                # group expsum; stage the row (bf16) and ship everything
                se = smalls.tile([128, 1], DT)
                nc.vector.tensor_reduce(
                    out=se, in_=estack[:, g0:g1],
                    axis=mybir.AxisListType.X, op=mybir.AluOpType.add)
                nc.gpsimd.partition_all_reduce(
                    sgrp[:, g:g + 1], se, 128, ReduceOp.add)
                nc.scalar.copy(stage[r:r + 1, :], psum_st[r:r + 1, :])
                row_out = bass.AP(tensor=cc_in.ap().tensor, offset=g * CCW,
                                  ap=[[0, 1], [1, Q]])
                nc.scalar.dma_start(out=row_out, in_=stage[r:r + 1, :])
                nc.scalar.dma_start(
                    out=cc_in[0:1, g * CCW + Q:g * CCW + Q + 2],
                    in_=mstack[0:1, g:g + 1].bitcast(BF))
                nc.scalar.dma_start(
                    out=cc_in[0:1, g * CCW + Q + 2:g * CCW + Q + 4],
                    in_=sgrp[0:1, g:g + 1].bitcast(BF))

        # ---- phase 2: K_w streams; query-half dots overlap everything ----
        accq = smalls.tile([128, RT], DT)
        accs = smalls.tile([128, RT], DT)
        acc = smalls.tile([128, RT], DT)
        kwqp = ctx.enter_context(tc.tile_pool(name="kwqp", bufs=2))
        kwsp = ctx.enter_context(tc.tile_pool(name="kwsp", bufs=2))

        def kw_pair(pool, p, col0):
            pair = pool.tile([128, 2, Q], DT)
            src = bass.AP(tensor=kw_ext.ap().tensor,
                          offset=p * 256 * 2 * Q + col0,
                          ap=[[2 * Q, 128], [128 * 2 * Q, 2], [1, Q]])
            nc.sync.dma_start(out=pair, in_=src)
            return pair

        kwq_tiles = []
        for p in range(RT // 2):
            pair = kw_pair(kwqp, p, 0)
            for h in range(2):
                j = 2 * p + h
                nc.vector.scalar_tensor_tensor(
                    out=dummy.broadcast_to([128, Q]),
                    in0=pair[:, h, :], scalar=1.0, in1=queryB,
                    op0=mybir.AluOpType.mult, op1=mybir.AluOpType.mult,
                    accum_out=accq[:, j:j + 1],
                )
        kws_tiles = []
        for p in range(RT // 2):
            pair = kw_pair(kwsp, p, Q)
            kws_tiles.extend([pair[:, 0, :], pair[:, 1, :]])

        # ---- phase 3: AllGather of the 24 bf16 group rows ----
        nc.gpsimd.collective_compute(
            "AllGather",
            mybir.AluOpType.bypass,
            replica_groups=[list(range(N_CORES))],
            ins=[cc_in.ap().opt()],
            outs=[cc_outA.ap().opt()],
        )
        gathered = late.tile([GROWS, CCW], BF)
        ginA = bass.AP(tensor=cc_outA.ap().tensor, offset=0,
                       ap=[[CCW, GROWS], [1, CCW]])
        nc.scalar.dma_start(out=gathered, in_=ginA)

        # ---- phase 4: global softmax combine, s_t broadcast into PSUM ----
        mg = gathered[:, Q:Q + 2].bitcast(DT)
        sg = gathered[:, Q + 2:Q + 4].bitcast(DT)
        mmax = smalls.tile([GROWS, 1], DT)
        nc.gpsimd.partition_all_reduce(mmax, mg, GROWS, ReduceOp.max)
        negM = smalls.tile([GROWS, 1], DT)
        nc.vector.tensor_scalar_mul(negM, mmax, -1.0)
        expm = smalls.tile([GROWS, 1], DT)
        nc.scalar.activation(out=expm, in_=mg,
                             func=mybir.ActivationFunctionType.Exp,
                             bias=negM, scale=1.0)
        w = smalls.tile([GROWS, 1], DT)
        nc.vector.tensor_mul(w, expm, sg)
        wsum = smalls.tile([GROWS, 1], DT)
        nc.gpsimd.partition_all_reduce(wsum, w, GROWS, ReduceOp.add)
        rS = smalls.tile([GROWS, 1], DT)
        nc.vector.reciprocal(rS, wsum)
        alpha = smalls.tile([GROWS, 1], DT)
        nc.vector.tensor_mul(alpha, expm, rS)
        alpha_rep = smalls.tile([GROWS, 128], BF)
        nc.vector.tensor_scalar_mul(alpha_rep, ones_rep, alpha)

        with tc.tile_pool(name="ps2", bufs=1, space="PSUM") as ps2:
            psum_stB = ps2.tile([128, Q], DT)
            for n in range(NB):
                sl = slice(n * 512, (n + 1) * 512)
                nc.tensor.matmul(
                    psum_stB[:, sl],
                    lhsT=alpha_rep,
                    rhs=gathered[0:GROWS, sl],
                    start=True, stop=True,
                )

            # ---- phase 5: K_w s_t-half dots against PSUM-resident s_t ----
            for j in range(RT):
                nc.vector.scalar_tensor_tensor(
                    out=dummy.broadcast_to([128, Q]),
                    in0=kws_tiles[j], scalar=1.0, in1=psum_stB,
                    op0=mybir.AluOpType.mult, op1=mybir.AluOpType.mult,
                    accum_out=accs[:, j:j + 1],
                )

        nc.vector.tensor_add(acc, accq, accs)
        nc.sync.dma_start(out=out_ext.ap(), in_=acc)

        if _DEBUG:
            nc.sync.dma_start(out=dbg_ext[0:1, 0:NG], in_=mstack[0:1, 0:NG])
            nc.sync.dma_start(out=dbg_ext[1:2, 0:NG], in_=sgrp[0:1, 0:NG])
            nc.sync.dma_start(out=dbg_ext[2:3, 0:1], in_=mg[0:1, 0:1])
            nc.sync.dma_start(out=dbg_ext[3:4, 0:8], in_=scores[0:1, 0:LT])
            nc.sync.dma_start(out=dbg_ext[4:5, 0:1], in_=alpha[0:1, 0:1])
            nc.sync.dma_start(out=dbg_ext[5:6, 0:1], in_=wsum[0:1, 0:1])
            nc.sync.dma_start(out=dbg_ext[6:7, 0:4], in_=accq[0:1, 0:4])
            nc.sync.dma_start(out=dbg_ext[7:8, 0:4], in_=accs[0:1, 0:4])

    nc.compile()
    return nc


def get_nc():
    if "nc" not in _NC_CACHE:
        _NC_CACHE["nc"] = build_nc()
    return _NC_CACHE["nc"]


def _shard_inputs(query, context_vector, K_w):
    qb = np.ascontiguousarray(
        np.broadcast_to(np.asarray(query, dtype=np.float32).reshape(1, Q),
                        (128, Q)))
    in_maps = []
    for c in range(N_CORES):
        in_maps.append({
            "query": qb,
            "cv": np.ascontiguousarray(
                context_vector[c * L_SHARD:(c + 1) * L_SHARD], dtype=np.float32),
            "kw": np.ascontiguousarray(
                K_w[c * R_SHARD:(c + 1) * R_SHARD], dtype=np.float32),
        })
    return in_maps


def kernel(query, context_vector, K_w, _trace=False, _trace_kwargs=None):
    nc = get_nc()
    in_maps = _shard_inputs(query, context_vector, K_w)
    res = run_bass_kernel_spmd(nc, in_maps, core_ids=list(range(N_CORES)),
                               trace=_trace, **(_trace_kwargs or {}))
    out = np.concatenate(
        [np.asarray(res.results[c]["out"]).T.reshape(-1) for c in range(N_CORES)]
    ).astype(np.float32)
    if _trace:
        kernel.last_results = res
    return out


# revision 24
# speedup vs baseline: 1.2032x; 1.0893x over previous
"""Distributed Trainium2 kernel for the attention GEMV chain:

    score = context_vector @ query            [L]         (L=8192, Q=4096)
    attn  = softmax(score)
    s_t   = attn @ context_vector             [Q]
    out   = K_w @ concat(query, s_t)          [Q]

Sharding over 8 NeuronCores:
  - context_vector rows: 1024 per core (score GEMV + partial weighted sums)
  - K_w rows: 512 per core (each core produces its own slice of the final
    output, so no output collective is needed)
  - flash-softmax partials are accumulated per GROUP of 3 cv tiles (group
    max as exp reference keeps exp <= 1, always fp32-safe); one AllGather
    moves the 3 bf16 group rows [s_t_grp(4096) | max | expsum] per core
    (fp32 stats ride along bit-cast into the bf16 payload); the global
    normalization finishes after the gather with an alpha-weighted rank-24
    bf16 matmul that also broadcasts s_t to all 128 partitions (in PSUM).

Per-core schedule (paced by the DMA stream):
  - query arrives pre-broadcast [128, 4096] from the host shard prep and
    loads on the scalar DMA ring, in parallel with cv tile 0 on sync.
  - per 128-row cv tile: fused mult+reduce (scalar_tensor_tensor) gives the
    128 scores in one DVE pass; the tile is cast to bf16 (alternating
    DVE/ACT); per group one gpsimd partition_all_reduce gives the maxes;
    ACT computes bf16 exp weights; TensorE accumulates the exp-weighted
    rows into the group's PSUM row (partitions {0,32,64}) with cheap bf16
    matmuls; one ACT copy per group stages the row (already bf16).
  - scores stay fp32 end-to-end (softmax is argmax-dominated); the weights
    and matrices tolerate bf16 (verified ~1e-3 rel err).
  - all SBUF pools are laid out disjoint (no stack reuse), so the K_w
    stream flows right behind the cv stream with no address-reuse stalls;
    K_w streams as 8 half-tiles [128, 4096], the query-half dot products
    run before/during the collective, the s_t-half ones read the broadcast
    s_t directly from PSUM.
"""
import sys

if "/opt/trn_rl_repo" not in sys.path:
    sys.path.insert(0, "/opt/trn_rl_repo")

from contextlib import ExitStack

import numpy as np

import concourse.bass as bass
import concourse.bacc as bacc
import concourse.mybir as mybir
import concourse.tile as tile
from concourse.bass_isa import ReduceOp
from concourse.bass_utils import run_bass_kernel_spmd

N_CORES = 8
Q = 4096
L = 8192
L_SHARD = L // N_CORES          # 1024 rows of context_vector per core
R_SHARD = Q // N_CORES          # 512 rows of K_w per core
LT = L_SHARD // 128             # 8 l-tiles per core
RT = R_SHARD // 128             # 4 r-tiles per core
NB = Q // 512                   # 8 psum banks of 512 fp32
CCW = Q + 16                    # bf16 collective row: s_t_grp, m, S, pad
GROUPS = [(0, 3), (3, 6), (6, 8)]   # cv tile groups, one PSUM row each
NG = len(GROUPS)
GROWS = N_CORES * NG            # 24 gathered rows
DT = mybir.dt.float32
BF = mybir.dt.bfloat16

_NC_CACHE = {}
_DEBUG = False


def build_nc():
    nc = bacc.Bacc("TRN2", target_bir_lowering=False, debug=False,
                   num_devices=N_CORES)

    q_ext = nc.dram_tensor("query", [128, Q], DT, kind="ExternalInput")
    cv_ext = nc.dram_tensor("cv", [L_SHARD, Q], DT, kind="ExternalInput")
    kw_ext = nc.dram_tensor("kw", [R_SHARD, 2 * Q], DT, kind="ExternalInput")
    out_ext = nc.dram_tensor("out", [128, RT], DT, kind="ExternalOutput")

    cc_in = nc.dram_tensor("cc_in", [1, NG * CCW], BF)
    cc_outA = nc.dram_tensor("cc_outA", [N_CORES, 2 * CCW], BF,
                             addr_space="Shared")
    cc_outB = nc.dram_tensor("cc_outB", [N_CORES, CCW], BF,
                             addr_space="Shared")
    dbg_ext = None
    if _DEBUG:
        dbg_ext = nc.dram_tensor("dbg", [16, 16], DT, kind="ExternalOutput")

    with tile.TileContext(nc) as tc, ExitStack() as ctx:
        persist = ctx.enter_context(tc.tile_pool(name="persist", bufs=1))
        smalls = ctx.enter_context(tc.tile_pool(name="smalls", bufs=1))
        late = ctx.enter_context(tc.tile_pool(name="late", bufs=1))

        queryB = persist.tile([128, Q], DT)
        nc.scalar.dma_start(out=queryB[:, 0:Q // 2], in_=q_ext[:, 0:Q // 2])
        nc.sync.dma_start(out=queryB[:, Q // 2:Q], in_=q_ext[:, Q // 2:Q])

        scores = smalls.tile([128, LT], DT)
        dummy = smalls.tile([128, 1], DT)
        mstack = smalls.tile([128, NG], DT)     # per-group max (replicated)
        nstack = smalls.tile([128, NG], DT)     # negated maxes
        estack = smalls.tile([128, LT], BF)     # per-tile bf16 exp weights
        sgrp = smalls.tile([128, NG], DT)       # per-group expsum
        tmp_max = smalls.tile([128, LT], DT)    # per-column partition maxes
        stage = persist.tile([128, Q], BF)      # staged bf16 rows {0,32,64}
        ones_rep = smalls.tile([GROWS, 128], BF)
        nc.vector.memset(ones_rep, 1.0)

        # ---- phase 1: stream cv; per-group scores, stats, weighted row ----
        with tc.tile_pool(name="cvp", bufs=3) as cvp, \
             tc.tile_pool(name="cvb", bufs=3) as cvb, \
             tc.tile_pool(name="ps1", bufs=1, space="PSUM") as ps1:
            psum_st = ps1.tile([128, Q], DT)
            for g, (g0, g1) in enumerate(GROUPS):
                r = 32 * g
                cvb_tiles = {}
                for t in range(g0, g1):
                    cv_t = cvp.tile([128, Q], DT)
                    nc.sync.dma_start(out=cv_t,
                                      in_=cv_ext[t * 128:(t + 1) * 128, :])
                    nc.vector.scalar_tensor_tensor(
                        out=dummy.broadcast_to([128, Q]),
                        in0=cv_t, scalar=1.0, in1=queryB,
                        op0=mybir.AluOpType.mult, op1=mybir.AluOpType.mult,
                        accum_out=scores[:, t:t + 1],
                    )
                    cvb_t = cvb.tile([128, Q], BF)
                    if t % 2 == 0:
                        nc.vector.tensor_copy(cvb_t, cv_t)
                    else:
                        nc.scalar.copy(cvb_t, cv_t)
                    cvb_tiles[t] = cvb_t
                # group stats: cross-partition max per column, group max,
                # negate, then per-tile exp weights
                nc.gpsimd.partition_all_reduce(
                    tmp_max[:, g0:g1], scores[:, g0:g1], 128, ReduceOp.max)
                nc.vector.tensor_reduce(
                    out=mstack[:, g:g + 1], in_=tmp_max[:, g0:g1],
                    axis=mybir.AxisListType.X, op=mybir.AluOpType.max)
                nc.vector.tensor_scalar_mul(
                    nstack[:, g:g + 1], mstack[:, g:g + 1], -1.0)
                for t in range(g0, g1):
                    nc.scalar.activation(
                        out=estack[:, t:t + 1], in_=scores[:, t:t + 1],
                        func=mybir.ActivationFunctionType.Exp,
                        bias=nstack[:, g:g + 1], scale=1.0)
                    for n in range(NB):
                        sl = slice(n * 512, (n + 1) * 512)
                        nc.tensor.matmul(
                            psum_st[r:r + 1, sl],
                            lhsT=estack[:, t:t + 1],
                            rhs=cvb_tiles[t][:, sl],
                            start=(t == g0), stop=(t == g1 - 1),
                            skip_group_check=True,
                        )
                # group expsum; stage the row (bf16) and ship everything
                se = smalls.tile([128, 1], DT)
                nc.vector.tensor_reduce(
                    out=se, in_=estack[:, g0:g1],
                    axis=mybir.AxisListType.X, op=mybir.AluOpType.add)
                nc.gpsimd.partition_all_reduce(
                    sgrp[:, g:g + 1], se, 128, ReduceOp.add)
                nc.scalar.copy(stage[r:r + 1, :], psum_st[r:r + 1, :])
                row_out = bass.AP(tensor=cc_in.ap().tensor, offset=g * CCW,
                                  ap=[[0, 1], [1, Q]])
                nc.scalar.dma_start(out=row_out, in_=stage[r:r + 1, :])
                nc.scalar.dma_start(
                    out=cc_in[0:1, g * CCW + Q:g * CCW + Q + 2],
                    in_=mstack[0:1, g:g + 1].bitcast(BF))
                nc.scalar.dma_start(
                    out=cc_in[0:1, g * CCW + Q + 2:g * CCW + Q + 4],
                    in_=sgrp[0:1, g:g + 1].bitcast(BF))

        # ---- phase 2: K_w streams; query-half dots overlap everything ----
        accq = smalls.tile([128, RT], DT)
        accs = smalls.tile([128, RT], DT)
        acc = smalls.tile([128, RT], DT)
        kwqp = ctx.enter_context(tc.tile_pool(name="kwqp", bufs=2))
        kwsp = ctx.enter_context(tc.tile_pool(name="kwsp", bufs=2))

        def kw_pair(pool, p, col0):
            pair = pool.tile([128, 2, Q], DT)
            src = bass.AP(tensor=kw_ext.ap().tensor,
                          offset=p * 256 * 2 * Q + col0,
                          ap=[[2 * Q, 128], [128 * 2 * Q, 2], [1, Q]])
            nc.sync.dma_start(out=pair, in_=src)
            return pair

        kwq_tiles = []
        for p in range(RT // 2):
            pair = kw_pair(kwqp, p, 0)
            for h in range(2):
                j = 2 * p + h
                nc.vector.scalar_tensor_tensor(
                    out=dummy.broadcast_to([128, Q]),
                    in0=pair[:, h, :], scalar=1.0, in1=queryB,
                    op0=mybir.AluOpType.mult, op1=mybir.AluOpType.mult,
                    accum_out=accq[:, j:j + 1],
                )
        kws_tiles = []
        for p in range(RT // 2):
            pair = kw_pair(kwsp, p, Q)
            kws_tiles.extend([pair[:, 0, :], pair[:, 1, :]])

        # ---- phase 3: split AllGather (ncfw runs them back to back) ----
        nc.gpsimd.collective_compute(
            "AllGather",
            mybir.AluOpType.bypass,
            replica_groups=[list(range(N_CORES))],
            ins=[cc_in[0:1, 0:2 * CCW].opt()],
            outs=[cc_outA.ap().opt()],
        )
        nc.gpsimd.collective_compute(
            "AllGather",
            mybir.AluOpType.bypass,
            replica_groups=[list(range(N_CORES))],
            ins=[cc_in[0:1, 2 * CCW:3 * CCW].opt()],
            outs=[cc_outB.ap().opt()],
        )
        gathered = late.tile([GROWS, CCW], BF)
        ginA = bass.AP(tensor=cc_outA.ap().tensor, offset=0,
                       ap=[[CCW, 16], [1, CCW]])
        nc.scalar.dma_start(out=gathered[0:16, :], in_=ginA)
        ginB = bass.AP(tensor=cc_outB.ap().tensor, offset=0,
                       ap=[[CCW, 8], [1, CCW]])
        nc.scalar.dma_start(out=gathered[16:24, :], in_=ginB)

        # ---- phase 4: global softmax combine, s_t broadcast into PSUM ----
        mg = gathered[:, Q:Q + 2].bitcast(DT)
        sg = gathered[:, Q + 2:Q + 4].bitcast(DT)
        mmax = smalls.tile([GROWS, 1], DT)
        nc.gpsimd.partition_all_reduce(mmax, mg, GROWS, ReduceOp.max)
        negM = smalls.tile([GROWS, 1], DT)
        nc.vector.tensor_scalar_mul(negM, mmax, -1.0)
        expm = smalls.tile([GROWS, 1], DT)
        nc.scalar.activation(out=expm, in_=mg,
                             func=mybir.ActivationFunctionType.Exp,
                             bias=negM, scale=1.0)
        w = smalls.tile([GROWS, 1], DT)
        nc.vector.tensor_mul(w, expm, sg)
        wsum = smalls.tile([GROWS, 1], DT)
        nc.gpsimd.partition_all_reduce(wsum, w, GROWS, ReduceOp.add)
        rS = smalls.tile([GROWS, 1], DT)
        nc.vector.reciprocal(rS, wsum)
        alpha = smalls.tile([GROWS, 1], DT)
        nc.vector.tensor_mul(alpha, expm, rS)
        alpha_rep = smalls.tile([GROWS, 128], BF)
        nc.vector.tensor_scalar_mul(alpha_rep, ones_rep, alpha)

        with tc.tile_pool(name="ps2", bufs=1, space="PSUM") as ps2:
            psum_stB = ps2.tile([128, Q], DT)
            for n in range(NB):
                sl = slice(n * 512, (n + 1) * 512)
                nc.tensor.matmul(
                    psum_stB[:, sl],
                    lhsT=alpha_rep,
                    rhs=gathered[0:GROWS, sl],
                    start=True, stop=True,
                )

            # ---- phase 5: K_w s_t-half dots against PSUM-resident s_t ----
            for j in range(RT):
                nc.vector.scalar_tensor_tensor(
                    out=dummy.broadcast_to([128, Q]),
                    in0=kws_tiles[j], scalar=1.0, in1=psum_stB,
                    op0=mybir.AluOpType.mult, op1=mybir.AluOpType.mult,
                    accum_out=accs[:, j:j + 1],
                )

        nc.vector.tensor_add(acc, accq, accs)
        nc.sync.dma_start(out=out_ext.ap(), in_=acc)

        if _DEBUG:
            nc.sync.dma_start(out=dbg_ext[0:1, 0:NG], in_=mstack[0:1, 0:NG])
            nc.sync.dma_start(out=dbg_ext[1:2, 0:NG], in_=sgrp[0:1, 0:NG])
            nc.sync.dma_start(out=dbg_ext[2:3, 0:1], in_=mg[0:1, 0:1])
            nc.sync.dma_start(out=dbg_ext[3:4, 0:8], in_=scores[0:1, 0:LT])
            nc.sync.dma_start(out=dbg_ext[4:5, 0:1], in_=alpha[0:1, 0:1])
            nc.sync.dma_start(out=dbg_ext[5:6, 0:1], in_=wsum[0:1, 0:1])
            nc.sync.dma_start(out=dbg_ext[6:7, 0:4], in_=accq[0:1, 0:4])
            nc.sync.dma_start(out=dbg_ext[7:8, 0:4], in_=accs[0:1, 0:4])

    nc.compile()
    return nc


def get_nc():
    if "nc" not in _NC_CACHE:
        _NC_CACHE["nc"] = build_nc()
    return _NC_CACHE["nc"]


def _shard_inputs(query, context_vector, K_w):
    qb = np.ascontiguousarray(
        np.broadcast_to(np.asarray(query, dtype=np.float32).reshape(1, Q),
                        (128, Q)))
    in_maps = []
    for c in range(N_CORES):
        in_maps.append({
            "query": qb,
            "cv": np.ascontiguousarray(
                context_vector[c * L_SHARD:(c + 1) * L_SHARD], dtype=np.float32),
            "kw": np.ascontiguousarray(
                K_w[c * R_SHARD:(c + 1) * R_SHARD], dtype=np.float32),
        })
    return in_maps


def kernel(query, context_vector, K_w, _trace=False, _trace_kwargs=None):
    nc = get_nc()
    in_maps = _shard_inputs(query, context_vector, K_w)
    res = run_bass_kernel_spmd(nc, in_maps, core_ids=list(range(N_CORES)),
                               trace=_trace, **(_trace_kwargs or {}))
    out = np.concatenate(
        [np.asarray(res.results[c]["out"]).T.reshape(-1) for c in range(N_CORES)]
    ).astype(np.float32)
    if _trace:
        kernel.last_results = res
    return out
